# revision 4
# baseline (speedup 1.0000x reference)
"""DeepseekV3 decoder layer on 8 Trainium2 NeuronCores (Bass/Tile).

Sharding: 8 cores = 4 batch elements x 2 sequence-halves.  Each core
computes the full layer for one batch element restricted to a
1024-query-token subset (two 512-token "zig-zag" stripes: half 0 owns
global stripes {0,3}, half 1 owns {1,2}), against the full 2048-token
key/value sequence.  No collectives; KV projections are recomputed per
core.  Attention FLOPs balance exactly across halves (20 key-blocks
each, padded to a uniform 24 so all 8 cores run one SPMD program).

All activations are feature-major ("T" suffix, [features, tokens]) so
every GEMM is out.T = W.T @ in.T with the contraction dim on SBUF
partitions, chaining naturally on the TensorEngine.  Matmuls run in
bf16 with fp32 accumulation; residual adds stay fp32.  Causal+padding
masking is additive mask data built on the host from the actual
attention_mask/positions inputs (softmax skips the max-subtraction:
scores are bounded for this problem's distribution, and masked lanes
use -30000 which underflows exp() to 0).

kernel(**inputs) takes the full unsharded inputs, shards on host,
runs one SPMD NEFF on cores 0-7 via run_bass_kernel_spmd, and gathers
the full [B, T, D] float32 output.
"""

import math
import os
import sys
from contextlib import ExitStack

import numpy as np

for _p in ("/opt/trn_rl_repo", "/root/.axon_site"):
    if _p not in sys.path and os.path.isdir(_p):
        sys.path.insert(0, _p)

import concourse.bass as bass
import concourse.bacc as bacc
import concourse.tile as tile
import concourse.mybir as mybir
from concourse.kernels.tile_matmul import matmul_tile_kernel
from concourse.masks import make_identity

P = 128
EPS = 1e-6
THETA = 10000.0
NEG = -30000.0  # additive mask value; exp() underflows to 0

F32 = mybir.dt.float32
BF16 = mybir.dt.bfloat16

CFG_FULL = dict(D=2048, T=2048, TQ=1024, H=16, QR=1536, KVR=512,
                NOPE=128, ROPE=64, VD=128, DFF=8192, KB=512)
B_FULL = 4

# global query stripes: 4 stripes of TQ/2 tokens; half 0 -> {0,3}, half 1 -> {1,2}
STRIPES_FOR_HALF = {0: (0, 3), 1: (1, 2)}


def stripe_width(cfg):
    return cfg["TQ"] // 2


def q_positions(cfg, half):
    sw = stripe_width(cfg)
    return np.concatenate([np.arange(s * sw, (s + 1) * sw)
                           for s in STRIPES_FOR_HALF[half]])


def blocks_per_stripe(cfg):
    sw = stripe_width(cfg)
    kb = cfg["KB"]
    out = []
    for sg in range(2):
        nb = 0
        for half in (0, 1):
            base = STRIPES_FOR_HALF[half][sg] * sw
            nb = max(nb, math.ceil((base + sw) / kb))
        out.append(nb)
    return out


def build_layer_nc(cfg, debug=False):
    D, T, TQ = cfg["D"], cfg["T"], cfg["TQ"]
    H, QR, KVR = cfg["H"], cfg["QR"], cfg["KVR"]
    NOPE, ROPE, VD, DFF, KB = (cfg["NOPE"], cfg["ROPE"], cfg["VD"],
                               cfg["DFF"], cfg["KB"])
    QK = NOPE + ROPE
    RH = ROPE // 2
    SW = stripe_width(cfg)
    NQS = SW // P
    NQT = TQ // P
    NB = blocks_per_stripe(cfg)
    SCALE = QK ** -0.5

    nc = bacc.Bacc(None, target_bir_lowering=False, debug=debug)
    io = {}

    with tile.TileContext(nc) as tc:
        est = ExitStack()
        with est:
            dram = est.enter_context(tc.tile_pool(name="dram_io", bufs=1, space="DRAM"))
            dtmp = est.enter_context(tc.tile_pool(name="dram_tmp", bufs=1, space="DRAM"))
            const = est.enter_context(tc.tile_pool(name="const", bufs=1))

            def din(name, shape, dt):
                t = dram.tile(shape, dt, kind="ExternalInput", name=name)
                io[name] = t.tensor.name
                return t

            xT = din("xT", [D, T], F32)
            xTq = din("xTq", [D, TQ], F32)
            cosq = din("cosq", [RH, TQ], BF16)
            sinq = din("sinq", [RH, TQ], BF16)
            cosk = din("cosk", [RH, T], BF16)
            sink = din("sink", [RH, T], BF16)
            maskq = din("maskq", [NQT, P, T], BF16)
            ln1_w = din("ln1_w", [D], F32)
            q_ln_w = din("q_ln_w", [QR], F32)
            kv_ln_w = din("kv_ln_w", [KVR], F32)
            ln2_w = din("ln2_w", [D], F32)
            wq_a = din("wq_a", [D, QR], BF16)
            wq_b = din("wq_b", [QR, H * QK], BF16)
            wkv_a_kv = din("wkv_a_kv", [D, KVR], BF16)
            wkv_a_r = din("wkv_a_r", [D, ROPE], BF16)
            wkv_b_k = din("wkv_b_k", [KVR, H * NOPE], BF16)
            wkv_b_v = din("wkv_b_v", [KVR, H * VD], BF16)
            wo = din("wo", [H * VD, D], BF16)
            w_gate = din("w_gate", [D, DFF], BF16)
            w_up = din("w_up", [D, DFF], BF16)
            w_down = din("w_down", [DFF, D], BF16)

            outT = dram.tile([D, TQ], F32, kind="ExternalOutput", name="outT")
            io["outT"] = outT.tensor.name

            hT = dtmp.tile([D, T], BF16, name="hT")
            hTq = dtmp.tile([D, TQ], BF16, name="hTq")
            q_aT = dtmp.tile([QR, TQ], BF16, name="q_aT")
            q_aTn = dtmp.tile([QR, TQ], BF16, name="q_aTn")
            qT = dtmp.tile([H * QK, TQ], BF16, name="qT")
            ckvT = dtmp.tile([KVR, T], BF16, name="ckvT")
            ckvnT = dtmp.tile([KVR, T], BF16, name="ckvnT")
            ckv_rT = dtmp.tile([ROPE, T], BF16, name="ckv_rT")
            kT = dtmp.tile([H * NOPE, T], BF16, name="kT")
            v_tm = dtmp.tile([T, H * VD], BF16, name="v_tm")
            oT = dtmp.tile([H * VD, TQ], BF16, name="oT")
            x2T = dtmp.tile([D, TQ], F32, name="x2T")
            h2T = dtmp.tile([D, TQ], BF16, name="h2T")
            gT = dtmp.tile([DFF, TQ], BF16, name="gT")
            mT = dtmp.tile([DFF, TQ], BF16, name="mT")

            ones_c = const.tile([P, 1], BF16, name="ones_c")
            nc.vector.memset(ones_c[:], 1.0)
            ones_r = const.tile([1, P], F32, name="ones_r")
            nc.vector.memset(ones_r[:], 1.0)
            ident_b = const.tile([P, P], BF16, name="ident_b")
            make_identity(nc, ident_b[:])
            ident_f = const.tile([P, P], F32, name="ident_f")
            make_identity(nc, ident_f[:])
            zeros_c = const.tile([P, 1], F32, name="zeros_c")
            nc.vector.memset(zeros_c[:], 0.0)
            eps_c = const.tile([P, 1], F32, name="eps_c")
            nc.vector.memset(eps_c[:], EPS)
            nc.const_aps.aps[(F32, 0.0)] = zeros_c[:]
            nc.const_aps.aps[(F32, EPS)] = eps_c[:]

            # ================= helpers =================

            def rmsnorm_T(src_ap, w_ap, out_ap, extra_scale=1.0, tag=""):
                """out = src * rsqrt(mean_col(src^2)+EPS) * w, feature-major."""
                F, Tt = src_ap.shape
                nF = F // P
                nJ = Tt // 512 if Tt >= 512 else 1
                JW = min(512, Tt)
                with ExitStack() as c2:
                    pl = c2.enter_context(tc.tile_pool(name=f"rms{tag}", bufs=3))
                    pp = c2.enter_context(
                        tc.tile_pool(name=f"rmsp{tag}", bufs=1, space="PSUM"))
                    w_sb = pl.tile([P, nF], F32, name=f"w_sb{tag}")
                    nc.sync.dma_start(w_sb[:], w_ap.rearrange("(o p) -> p o", p=P))
                    ss = pp.tile([1, Tt], F32, name=f"ss{tag}")
                    for i in range(nF):
                        t = pl.tile([P, Tt], src_ap.dtype, name=f"src{tag}")
                        nc.sync.dma_start(t[:], src_ap[i * P:(i + 1) * P, :])
                        sq = pl.tile([P, Tt], BF16, name=f"sq{tag}")
                        nc.vector.tensor_mul(sq[:], t[:], t[:])
                        for j in range(nJ):
                            nc.tensor.matmul(
                                ss[0:1, j * JW:(j + 1) * JW],
                                ones_c[:, 0:1], sq[:, j * JW:(j + 1) * JW],
                                start=(i == 0), stop=(i == nF - 1))
                    rms = pl.tile([1, Tt], F32, name=f"rms{tag}")
                    nc.scalar.activation(rms[:], ss[:],
                                         mybir.ActivationFunctionType.Sqrt,
                                         bias=EPS, scale=1.0 / F)
                    s1 = pl.tile([1, Tt], F32, name=f"s1{tag}")
                    nc.vector.reciprocal(s1[:], rms[:])
                    if extra_scale != 1.0:
                        nc.vector.tensor_scalar_mul(s1[:], s1[:], float(extra_scale))
                    sb = pp.tile([P, Tt], F32, name=f"sbp{tag}")
                    for j in range(nJ):
                        nc.tensor.matmul(sb[:, j * JW:(j + 1) * JW],
                                         ones_r[:], s1[0:1, j * JW:(j + 1) * JW])
                    s1b = pl.tile([P, Tt], F32, name=f"s1b{tag}")
                    nc.scalar.copy(s1b[:], sb[:])
                    for i in range(nF):
                        t = pl.tile([P, Tt], src_ap.dtype, name=f"src2{tag}")
                        nc.sync.dma_start(t[:], src_ap[i * P:(i + 1) * P, :])
                        o = pl.tile([P, Tt], BF16, name=f"o{tag}")
                        nc.vector.scalar_tensor_tensor(
                            o[:], t[:], w_sb[:, i:i + 1], s1b[:],
                            op0=mybir.AluOpType.mult, op1=mybir.AluOpType.mult)
                        nc.sync.dma_start(out_ap[i * P:(i + 1) * P, :], o[:])

            def rope_T(src_gather_ap, cos_t, sin_t, hh, Tt, dst_gather_ap=None,
                       dst_sb=None, tag=""):
                """Interleaved-half RoPE over hh head-chunks of ROPE rows.

                All compute tiles live at base partition 0 (hardware
                requires equal base partitions for 2-input DVE ops);
                results move to their final rows via DMA."""
                with ExitStack() as c2:
                    pl = c2.enter_context(tc.tile_pool(name=f"rope{tag}", bufs=2))

                    def part(ap, r0, c0, hc):
                        return bass.AP(
                            tensor=ap.tensor,
                            offset=ap.offset + r0 * ap.ap[0][0] + c0 * ap.ap[1][0],
                            ap=[[ap.ap[0][0], RH], [ap.ap[1][0], hc], ap.ap[2]])

                    HC = min(4, hh)
                    for c0 in range(0, hh, HC):
                        x1 = pl.tile([RH, HC, Tt], BF16, name=f"x1{tag}")
                        x2 = pl.tile([RH, HC, Tt], BF16, name=f"x2{tag}")
                        nc.sync.dma_start(x1[:], part(src_gather_ap, 0, c0, HC))
                        nc.sync.dma_start(x2[:], part(src_gather_ap, RH, c0, HC))

                        def bc(ap):
                            return bass.AP(tensor=ap.tensor, offset=ap.offset,
                                           ap=[ap.ap[0], [0, HC], ap.ap[1]])

                        c = bc(cos_t[:])
                        s = bc(sin_t[:])
                        olo = pl.tile([RH, HC, Tt], BF16, name=f"ol{tag}")
                        ohi = pl.tile([RH, HC, Tt], BF16, name=f"oh{tag}")
                        tmp = pl.tile([RH, HC, Tt], BF16, name=f"tm{tag}")
                        nc.vector.tensor_mul(tmp[:], x2[:], s)
                        nc.vector.tensor_mul(olo[:], x1[:], c)
                        nc.vector.tensor_tensor(olo[:], olo[:], tmp[:],
                                                op=mybir.AluOpType.subtract)
                        tmp2 = pl.tile([RH, HC, Tt], BF16, name=f"tn{tag}")
                        nc.vector.tensor_mul(tmp2[:], x1[:], s)
                        nc.vector.tensor_mul(ohi[:], x2[:], c)
                        nc.vector.tensor_tensor(ohi[:], ohi[:], tmp2[:],
                                                op=mybir.AluOpType.add)
                        if dst_gather_ap is not None:
                            nc.sync.dma_start(part(dst_gather_ap, 0, c0, HC),
                                              olo[:])
                            nc.sync.dma_start(part(dst_gather_ap, RH, c0, HC),
                                              ohi[:])
                        if dst_sb is not None:  # SBUF tile [2*RH, Tt], hh==1
                            nc.sync.dma_start(dst_sb[0:RH, :], olo[:, 0, :])
                            nc.sync.dma_start(dst_sb[RH:2 * RH, :], ohi[:, 0, :])

            # ============ phase 1: norms + projections ============

            rmsnorm_T(xT[:], ln1_w[:], hT[:], tag="h")
            rmsnorm_T(xTq[:], ln1_w[:], hTq[:], tag="hq")

            matmul_tile_kernel(tc, wq_a[:], hTq[:], q_aT[:])
            rmsnorm_T(q_aT[:], q_ln_w[:], q_aTn[:], extra_scale=SCALE, tag="qa")
            matmul_tile_kernel(tc, wq_b[:], q_aTn[:], qT[:])

            matmul_tile_kernel(tc, wkv_a_kv[:], hT[:], ckvT[:])
            matmul_tile_kernel(tc, wkv_a_r[:], hT[:], ckv_rT[:])
            rmsnorm_T(ckvT[:], kv_ln_w[:], ckvnT[:], tag="kv")
            matmul_tile_kernel(tc, wkv_b_k[:], ckvnT[:], kT[:])
            matmul_tile_kernel(tc, ckvnT[:], wkv_b_v[:], v_tm[:])

            # ---- RoPE ----
            with ExitStack() as c2:
                cpool = c2.enter_context(tc.tile_pool(name="ropec", bufs=1))
                cq = cpool.tile([RH, TQ], BF16, name="cq")
                sq_ = cpool.tile([RH, TQ], BF16, name="sq_")
                ck = cpool.tile([RH, T], BF16, name="ck")
                sk = cpool.tile([RH, T], BF16, name="sk")
                nc.sync.dma_start(cq[:], cosq[:])
                nc.sync.dma_start(sq_[:], sinq[:])
                nc.sync.dma_start(ck[:], cosk[:])
                nc.sync.dma_start(sk[:], sink[:])

                qT_ap = qT[:]
                gather = bass.AP(
                    tensor=qT_ap.tensor, offset=qT_ap.offset + NOPE * TQ,
                    ap=[[TQ, ROPE], [QK * TQ, H], [1, TQ]])
                rope_T(gather, cq, sq_, H, TQ, dst_gather_ap=gather, tag="q")

                krot = const.tile([ROPE, T], BF16, name="krot")
                ckr_ap = ckv_rT[:]
                kgather = bass.AP(tensor=ckr_ap.tensor, offset=ckr_ap.offset,
                                  ap=[[T, ROPE], [0, 1], [1, T]])
                rope_T(kgather, ck, sk, 1, T, dst_sb=krot[:], tag="k")

            # ============ phase 2: attention ============

            with ExitStack() as c2:
                apool = c2.enter_context(tc.tile_pool(name="attn_in", bufs=2))
                ppool = c2.enter_context(tc.tile_pool(name="attn_probs", bufs=2))
                tpool = c2.enter_context(tc.tile_pool(name="attn_pt", bufs=6))
                opool = c2.enter_context(tc.tile_pool(name="attn_out", bufs=3))
                mpool = c2.enter_context(tc.tile_pool(name="attn_mask", bufs=1))
                ps_s = c2.enter_context(
                    tc.tile_pool(name="ps_s", bufs=2, space="PSUM"))
                ps_t = c2.enter_context(
                    tc.tile_pool(name="ps_t", bufs=2, space="PSUM"))
                ps_o = c2.enter_context(
                    tc.tile_pool(name="ps_o", bufs=1, space="PSUM"))
                ps_r = c2.enter_context(
                    tc.tile_pool(name="ps_r", bufs=1, space="PSUM"))

                mask_sb = mpool.tile([P, NQT, T], BF16, name="mask_sb")
                nc.sync.dma_start(mask_sb[:], maskq[:].rearrange("q p t -> p q t"))

                for h in range(H):
                    kA = apool.tile([P, T], BF16, name="kA")
                    nc.sync.dma_start(kA[:], kT[h * NOPE:(h + 1) * NOPE, :])
                    qA = apool.tile([P, TQ], BF16, name="qA")
                    nc.sync.dma_start(qA[:], qT[h * QK:h * QK + NOPE, :])
                    qB = apool.tile([ROPE, TQ], BF16, name="qB")
                    nc.sync.dma_start(qB[:], qT[h * QK + NOPE:(h + 1) * QK, :])
                    v_sb = apool.tile([P, T // P, VD], BF16, name="v_sb")
                    vap = v_tm[:]
                    nc.sync.dma_start(v_sb[:], bass.AP(
                        tensor=vap.tensor, offset=vap.offset + h * VD,
                        ap=[[H * VD, P], [P * H * VD, T // P], [1, VD]]))

                    for sg in range(2):
                        nb = NB[sg]
                        nk = nb * KB // P
                        probs = []
                        rows = opool.tile([P, NQS, nb], F32, name="rows")
                        for ql in range(NQS):
                            qt = sg * NQS + ql
                            pr = ppool.tile([P, nb * KB], BF16, name=f"pr{ql}")
                            probs.append(pr)
                            pss = ps_s.tile([P, KB], F32, name="pss")
                            for kb in range(nb):
                                nc.tensor.matmul(
                                    pss[:], qA[:, qt * P:(qt + 1) * P],
                                    kA[:, kb * KB:(kb + 1) * KB],
                                    start=True, stop=False)
                                nc.tensor.matmul(
                                    pss[:], qB[:, qt * P:(qt + 1) * P],
                                    krot[:, kb * KB:(kb + 1) * KB],
                                    start=False, stop=True)
                                nc.vector.tensor_tensor(
                                    pss[:], pss[:],
                                    mask_sb[:, qt, kb * KB:(kb + 1) * KB],
                                    op=mybir.AluOpType.add)
                                nc.scalar.activation(
                                    pr[:, kb * KB:(kb + 1) * KB], pss[:],
                                    mybir.ActivationFunctionType.Exp,
                                    accum_out=rows[:, ql, kb:kb + 1])
                        pts = []
                        for ks in range(nk):
                            ptp = ps_t.tile([P, NQS * P], BF16, name="ptp")
                            for ql in range(NQS):
                                nc.tensor.transpose(
                                    ptp[:, ql * P:(ql + 1) * P],
                                    probs[ql][:, ks * P:(ks + 1) * P],
                                    ident_b[:])
                            pt = tpool.tile([P, NQS * P], BF16, name="pt")
                            nc.scalar.copy(pt[:], ptp[:])
                            pts.append(pt)
                        po = ps_o.tile([P, SW], F32, name="po")
                        for ks in range(nk):
                            nc.tensor.matmul(
                                po[:, 0:SW], v_sb[:, ks, :], pts[ks][:],
                                start=(ks == 0), stop=(ks == nk - 1))
                        rs = opool.tile([P, NQS], F32, name="rs")
                        nc.vector.reduce_sum(rs[:], rows[:],
                                             axis=mybir.AxisListType.X)
                        prt = ps_r.tile([1, SW], F32, name="prt")
                        for ql in range(NQS):
                            nc.tensor.transpose(prt[0:1, ql * P:(ql + 1) * P],
                                                rs[:, ql:ql + 1], ident_f[:])
                        rinv = opool.tile([1, SW], F32, name="rinv")
                        nc.vector.reciprocal(rinv[:], prt[0:1, 0:SW])
                        prb = ps_r.tile([P, SW], F32, name="prb")
                        nc.tensor.matmul(prb[:], ones_r[:], rinv[:],
                                         start=True, stop=True)
                        rb = opool.tile([P, SW], F32, name="rb")
                        nc.scalar.copy(rb[:], prb[:])
                        ot = opool.tile([P, SW], BF16, name="ot")
                        nc.vector.tensor_mul(ot[:], po[:, 0:SW], rb[:])
                        nc.sync.dma_start(
                            oT[h * VD:(h + 1) * VD, sg * SW:(sg + 1) * SW], ot[:])

            # ============ phase 3: output proj + MLP ============

            with ExitStack() as c2:
                ac6 = c2.enter_context(tc.tile_pool(name="acc6", bufs=2))
                matmul_tile_kernel(tc, wo[:], oT[:], x2T[:],
                                   accumulate_ap=xTq[:], accum_pool=ac6)
            rmsnorm_T(x2T[:], ln2_w[:], h2T[:], tag="h2")
            matmul_tile_kernel(tc, w_gate[:], h2T[:], gT[:])

            with ExitStack() as c2:
                fpool = c2.enter_context(tc.tile_pool(name="fuse", bufs=3))

                def fuse_silu(nc_, sbuf, md, _):
                    m0 = md.m_tile_idx * md.m_tile
                    n0 = md.n_tile_idx * md.n_tile
                    msub = sbuf.shape[1]
                    nsz = sbuf.shape[2]
                    g = fpool.tile([P, msub, nsz], BF16, name="gtile")
                    gv = gT[:].rearrange("(o p) t -> p o t", p=P)
                    nc_.sync.dma_start(
                        g[:], gv[:, m0 // P:m0 // P + msub, n0:n0 + nsz])
                    gs = fpool.tile([P, msub, nsz], BF16, name="gsil")
                    nc_.scalar.activation(gs[:], g[:],
                                          mybir.ActivationFunctionType.Sigmoid)
                    nc_.vector.tensor_mul(gs[:], gs[:], g[:])
                    nc_.vector.tensor_mul(sbuf[:], sbuf[:], gs[:])

                matmul_tile_kernel(tc, w_up[:], h2T[:], mT[:],
                                   post_mxn_tile_fn=fuse_silu)

            with ExitStack() as c2:
                ac9 = c2.enter_context(tc.tile_pool(name="acc9", bufs=2))
                matmul_tile_kernel(tc, w_down[:], mT[:], outT[:],
                                   accumulate_ap=x2T[:], accum_pool=ac9)

    nc.compile()
    return nc, io


# ===================== host-side glue =====================

def make_shared_weights(cfg, inputs):
    import ml_dtypes
    bf16 = ml_dtypes.bfloat16
    KVR, NOPE, VD, H = cfg["KVR"], cfg["NOPE"], cfg["VD"], cfg["H"]
    f32 = np.float32
    wkv_a = np.asarray(inputs["wkv_a"], f32)
    wkv_b = np.asarray(inputs["wkv_b"], f32).reshape(KVR, H, NOPE + VD)
    return dict(
        ln1_w=np.asarray(inputs["ln1_w"], f32),
        q_ln_w=np.asarray(inputs["q_a_ln_w"], f32),
        kv_ln_w=np.asarray(inputs["kv_a_ln_w"], f32),
        ln2_w=np.asarray(inputs["ln2_w"], f32),
        wq_a=np.asarray(inputs["wq_a"], f32).astype(bf16),
        wq_b=np.asarray(inputs["wq_b"], f32).astype(bf16),
        wkv_a_kv=np.ascontiguousarray(wkv_a[:, :KVR]).astype(bf16),
        wkv_a_r=np.ascontiguousarray(wkv_a[:, KVR:]).astype(bf16),
        wkv_b_k=np.ascontiguousarray(
            wkv_b[:, :, :NOPE].reshape(KVR, H * NOPE)).astype(bf16),
        wkv_b_v=np.ascontiguousarray(
            wkv_b[:, :, NOPE:].reshape(KVR, H * VD)).astype(bf16),
        wo=np.asarray(inputs["wo"], f32).astype(bf16),
        w_gate=np.asarray(inputs["w_gate"], f32).astype(bf16),
        w_up=np.asarray(inputs["w_up"], f32).astype(bf16),
        w_down=np.asarray(inputs["w_down"], f32).astype(bf16),
    )


def make_core_inputs(cfg, io, x_b, mask_b, pos_b, shared):
    """Input maps for the two halves of one batch element."""
    import ml_dtypes
    bf16 = ml_dtypes.bfloat16
    T, TQ, ROPE = cfg["T"], cfg["TQ"], cfg["ROPE"]
    NQT = TQ // P

    inv_freq = 1.0 / (THETA ** (np.arange(0, ROPE, 2, dtype=np.float32) / ROPE))
    ang = pos_b.astype(np.float32)[:, None] * inv_freq[None, :]
    cosT = np.cos(ang).T.astype(bf16)
    sinT = np.sin(ang).T.astype(bf16)

    xT = np.ascontiguousarray(x_b.T.astype(np.float32))

    maps = []
    for half in (0, 1):
        qpos = q_positions(cfg, half)
        xTq = np.ascontiguousarray(xT[:, qpos])
        gq = qpos.reshape(NQT, P)
        allowed = (np.arange(T)[None, None, :] <= gq[:, :, None]) \
            & mask_b[None, None, :]
        m = np.where(allowed, np.float32(0), np.float32(NEG)).astype(bf16)
        im = {
            io["xT"]: xT,
            io["xTq"]: xTq,
            io["cosq"]: np.ascontiguousarray(cosT[:, qpos]),
            io["sinq"]: np.ascontiguousarray(sinT[:, qpos]),
            io["cosk"]: cosT,
            io["sink"]: sinT,
            io["maskq"]: m,
        }
        for k in ("ln1_w", "q_ln_w", "kv_ln_w", "ln2_w", "wq_a", "wq_b",
                  "wkv_a_kv", "wkv_a_r", "wkv_b_k", "wkv_b_v", "wo",
                  "w_gate", "w_up", "w_down"):
            im[io[k]] = shared[k]
        maps.append(im)
    return maps


_CACHE = {}
LAST_EXEC_NS = None
LAST_TRACE = None


def _get_program():
    if "nc" not in _CACHE:
        _CACHE["nc"], _CACHE["io"] = build_layer_nc(CFG_FULL, debug=False)
    return _CACHE["nc"], _CACHE["io"]


def _install_profile_shim():
    """Register the axon NTFF profiling hook if the image lacks it."""
    import types
    try:
        from antenv.axon_hooks import get_axon_ntff_profile_hook  # noqa: F401
        return
    except ImportError:
        pass
    try:
        import antenv
        mod = types.ModuleType('antenv.axon_hooks')
        _hook = [None]
        mod.set_axon_ntff_profile_hook = lambda h: _hook.__setitem__(0, h)
        mod.get_axon_ntff_profile_hook = lambda: _hook[0]
        sys.modules['antenv.axon_hooks'] = mod
        antenv.axon_hooks = mod
        from trn_agent_boot.trn_boot import _ntff_profile_via_ctypes
        mod.set_axon_ntff_profile_hook(
            _ntff_profile_via_ctypes('/opt/axon/libaxon_pjrt.so'))
    except Exception:
        pass


def kernel(**inputs):
    global LAST_EXEC_NS, LAST_TRACE
    from concourse.bass_utils import run_bass_kernel_spmd

    cfg = CFG_FULL
    B, T, D = B_FULL, cfg["T"], cfg["D"]

    x = np.asarray(inputs["x"], np.float32)
    attention_mask = np.asarray(inputs["attention_mask"]).astype(bool)
    positions = np.asarray(inputs["positions"])

    nc, io = _get_program()
    shared = make_shared_weights(cfg, inputs)

    in_maps = []
    for b in range(B):
        in_maps.extend(make_core_inputs(cfg, io, x[b], attention_mask[b],
                                        positions[b], shared))

    trace = bool(int(os.environ.get("TRN_KERNEL_PROFILE", "0")))
    if trace:
        _install_profile_shim()
    res = run_bass_kernel_spmd(nc, in_maps, core_ids=list(range(2 * B)),
                               trace=trace)
    LAST_EXEC_NS = res.exec_time_ns
    LAST_TRACE = (res.instructions_and_trace[1]
                  if res.instructions_and_trace else None)

    out = np.empty((B, T, D), np.float32)
    for b in range(B):
        for half in (0, 1):
            outT = np.asarray(res.results[b * 2 + half][io["outT"]])
            out[b, q_positions(cfg, half), :] = outT.T
    return out
